# revision 70
# baseline (speedup 1.0000x reference)
"""AttentionalPooler Trainium2 kernel (fp16 PE paths, LN folded into matmuls).

Data-parallel over batch: each of 8 NeuronCores processes one batch element
(x[i]: [4096, 1024], shipped to the device pre-cast to fp16).

Key design points vs the v0 fp16 baseline (331us):
  - LayerNorm of x is folded into the matmuls instead of materializing
    z = (x-mu)*r on the vector engine:
      * the K weights carry a constant -1/1024 column, so the K projection
        psum row 96 is exactly -mu_j (computed by the PE for free);
      * sim accumulates q.kraw - mu_j*t_i via augmented 97-row operands,
        where t_i = q_i . colsum(W_k) is folded into an extra W_q output
        column on the host;
      * the Exp activation applies the per-token 1/std scale r_j
        (per-partition scale AP) -- exp(r*(q.kraw - mu*t)) == exp(q.k_ln);
      * the V psum drain applies r_j the same way, and the -mu*colsum(W_v)
        correction rides the attention matmul as an extra v_aug column
        (rmu), normalized with the softmax denominator and cancelled at the
        output projection through an extra -csv@W_out row of W_out.
    This removes 32 big DVE ops per call and unchains transposes from LN:
    transposes consume raw x tiles straight off the DMA.
  - A ~4.5us warmup burst of dummy matmuls at kernel start flips the PE
    HAM clock gate to 8/8 before real work lands and fills the DMA wait.
  - All matmul operands fp16; LN gain and the dh^-0.5 scale folded into
    weights on the host; K-path bias dropped (softmax invariant); V bias
    folded into an output-side bias add.
  - rsqrt(var+eps) via Newton iteration on the vector engine (the scalar
    engine only runs EXP and psum drains).
  - Software-pipelined chunks as in v0: head h's K matmuls cover head
    h-1's sim psum drains; next chunk's stats and this chunk's transposes
    are dripped into the head loop.
"""

import sys

for p in ("/opt/trn_rl_repo",):
    if p not in sys.path:
        sys.path.insert(0, p)

import numpy as np

import concourse.bass as bass
import concourse.tile as tile
from concourse import bacc
from concourse import mybir
from concourse.bass_utils import run_bass_kernel_spmd

F32 = mybir.dt.float32
F16 = mybir.dt.float16
I32 = mybir.dt.int32

N_CORES = 8
B, N, CTX = 8, 4096, 1024
NQ, DM, H = 256, 768, 8
DH = DM // H  # 96
DHA = DH + 1  # 97: projections carry one augmented row/col
DHV = DH + 2  # 98: v_aug adds rmu + ones columns
EPS = 1e-5
CHUNK = 512
N_CHUNKS = N // CHUNK  # 8
RT = CHUNK // 128  # 4 row-tiles per chunk
CC = CTX // 128  # 8 contraction chunks
RSQRT_MAGIC = 0x5F3759DF
WKA = H * DHA  # 776: augmented K-weight width


def emit_rsqrt(nc, pool, out, var, eps, tag):
    """out = 1/sqrt(var + eps), [128, n] f32, vector engine only."""
    n = out.shape[1]
    ve = pool.tile([128, n], F32, tag=f"{tag}_ve")
    nc.vector.tensor_scalar_add(out=ve, in0=var, scalar1=float(eps))
    sh = pool.tile([128, n], I32, tag=f"{tag}_sh")
    nc.vector.tensor_scalar(
        out=sh, in0=ve.bitcast(I32), scalar1=1, scalar2=None,
        op0=mybir.AluOpType.logical_shift_right,
    )
    magic = pool.tile([128, n], I32, tag=f"{tag}_mg")
    nc.vector.memset(magic, RSQRT_MAGIC)
    # y0 = bitcast(magic - (i >> 1))
    nc.vector.scalar_tensor_tensor(
        out=out.bitcast(I32), in0=magic, scalar=0, in1=sh,
        op0=mybir.AluOpType.bypass, op1=mybir.AluOpType.subtract,
    )
    t = pool.tile([128, n], F32, tag=f"{tag}_t")
    for _ in range(2):  # two Newton steps: rel err ~4e-6
        nc.vector.tensor_mul(out=t, in0=out, in1=out)
        nc.vector.tensor_mul(out=t, in0=t, in1=ve)
        nc.vector.tensor_scalar(
            out=t, in0=t, scalar1=-0.5, scalar2=1.5,
            op0=mybir.AluOpType.mult, op1=mybir.AluOpType.add,
        )
        nc.vector.tensor_mul(out=out, in0=out, in1=t)


def build_nc(has_out_bias):
    nc = bacc.Bacc("TRN2", debug=False)
    x = nc.dram_tensor("x", [N, CTX], F16, kind="ExternalInput")
    query = nc.dram_tensor("query", [NQ, DM], F16, kind="ExternalInput")
    wkvg = nc.dram_tensor("wkvg", [CTX, WKA + DM], F16, kind="ExternalInput")
    wqg = nc.dram_tensor("wqg", [DM, WKA], F16, kind="ExternalInput")
    wout = nc.dram_tensor("wout", [WKA, DM], F16, kind="ExternalInput")
    qbias = nc.dram_tensor("qbias", [WKA], F32, kind="ExternalInput")
    identh_p = nc.dram_tensor("identh", [128, 128], F16, kind="ExternalInput")
    identf_p = nc.dram_tensor("identf", [128, 128], F32, kind="ExternalInput")
    if has_out_bias:
        out_bias = nc.dram_tensor("out_bias", [1, DM], F32, kind="ExternalInput")
    out = nc.dram_tensor("out", [NQ, DM], F32, kind="ExternalOutput")

    from contextlib import ExitStack

    with tile.TileContext(nc) as tc, ExitStack() as es:
        singles = es.enter_context(tc.tile_pool(name="singles", bufs=1))
        work = es.enter_context(tc.tile_pool(name="work", bufs=3))
        xpool = es.enter_context(tc.tile_pool(name="xpool", bufs=4))
        chunkp = es.enter_context(tc.tile_pool(name="chunkp", bufs=2))
        # zT is DMA'd (xbar transpose) one loop ahead and read through the
        # consuming loop -> 3-deep ring
        ztp = es.enter_context(tc.tile_pool(name="ztp", bufs=4))
        # stats live one full head-loop longer than in the zrow world (Exp
        # scale and v_aug fills read them) -- deeper ring so the dripped
        # next-chunk stats never block on this chunk's readers
        statp = es.enter_context(tc.tile_pool(name="statp", bufs=4))
        # bufs=8: loop 1 defers attn@v past the head loop, keeping all 8
        # heads' at4 tiles live at once
        attp = es.enter_context(tc.tile_pool(name="attp", bufs=8))
        # PSUM pools: bank-granular; 1+2+2+2+1 = 8 banks exactly
        pz_pool = es.enter_context(tc.tile_pool(name="pz", bufs=1, space="PSUM"))
        pk_pool = es.enter_context(tc.tile_pool(name="pk", bufs=2, space="PSUM"))
        pv_pool = es.enter_context(tc.tile_pool(name="pv", bufs=2, space="PSUM"))
        ps_pool = es.enter_context(tc.tile_pool(name="ps", bufs=2, space="PSUM"))
        pa_pool = es.enter_context(tc.tile_pool(name="pa", bufs=1, space="PSUM"))

        # NOTE: no PE warmup burst -- measured on this part, any added PE
        # duty density early in the kernel trips the power-state downclock
        # (2.4 -> 2.0 GHz for the rest of the kernel), costing far more
        # than the HAM cold-clock it would have saved.

        # ---- DMAs. Two HW queues. Critical path to K(0) = zT(0) + wkv-K,
        # split ACROSS the queues so both land ~12us:
        #   SYNC: identh -> xbar0/xnat0 -> xbar1/xnat1 -> xbar2..7 -> wout
        #   ACT:  wkv-K -> qtiles -> wq -> qb -> wkv-V -> identf
        identh = singles.tile([128, 128], F16)
        nc.sync.dma_start(out=identh, in_=identh_p[:, :])
        qtiles = singles.tile([128, 2, DM], F16)
        nc.scalar.dma_start(out=qtiles[:, 0, :], in_=query[0:128, :])
        nc.scalar.dma_start(out=qtiles[:, 1, :], in_=query[128:256, :])
        zTs, vaugs = {}, {}
        ln_state = {}

        def emit_ln_alloc(c):
            mv_all = statp.tile([128, RT, 2], F32, tag="mv")
            rstd4 = statp.tile([128, RT], F32, tag="rstd")
            zT_c = ztp.tile([128, CC, CHUNK], F16, tag="zT")
            # zT straight from HBM via the DMA xbar transpose (14ns per
            # 16x128 tile = ~3.6us/chunk of DMA-engine time): no PE
            # transposes, no DVE psum drains. x is (cheaply) loaded twice:
            # natural for the LN stats, transposed for the matmuls. Both on
            # the (otherwise idle) sync queue, one DMA each, so the shared
            # DMA semaphore pool never throttles the stream.
            if c > 0:
                nc.sync.dma_start_transpose(
                    out=zT_c, in_=x[c * CHUNK : (c + 1) * CHUNK, :]
                )
            # chunk 0's zT is built by PE transposes instead (below): the PE
            # is otherwise idle while wkv-K streams, and it takes zT(0) off
            # the K(0) critical path
            xts = xpool.tile([128, RT, CTX], F16, tag="xt")
            nc.sync.dma_start(
                out=xts,
                in_=x[c * CHUNK : (c + 1) * CHUNK, :].rearrange(
                    "(rt p) ctx -> p rt ctx", p=128
                ),
            )
            ln_state[c] = (xts, mv_all, rstd4)
            zTs[c] = zT_c

        # alloc chunks 0/1 here so their xbar-transpose + natural loads
        # lead the sync queue (zT(0) gates K(0))
        emit_ln_alloc(0)
        wkv_sb = singles.tile([128, CC, WKA + DM], F16)
        wkv_re = wkvg.rearrange("(cc p) j -> p cc j", p=128)
        nc.sync.dma_start(out=wkv_sb[:, :, 0:WKA], in_=wkv_re[:, :, 0:WKA])
        emit_ln_alloc(1)
        wq_sb = singles.tile([128, DM // 128, WKA], F16)
        nc.scalar.dma_start(out=wq_sb, in_=wqg.rearrange("(cc p) j -> p cc j", p=128))
        qb_sb = singles.tile([DHA, H], F32)
        nc.scalar.dma_start(out=qb_sb, in_=qbias.rearrange("(h p) -> p h", p=DHA))
        nc.scalar.dma_start(out=wkv_sb[:, :, WKA:], in_=wkv_re[:, :, WKA:])
        identf = singles.tile([128, 128], F32)
        nc.scalar.dma_start(out=identf, in_=identf_p[:, :])
        wout_sb = singles.tile([DHA, H, DM], F16)
        # prefetch the WHOLE x stream now: chunks 2..7's DMA issues queue
        # behind the critical preamble transfers and simply park the sync
        # engine on their ring-WAR (which resolves a loop before the data
        # is needed). Nothing recycles a semaphore held by a big weight DMA
        # mid-kernel, which was worth ~30us of convoy stalls.
        for c_pre in range(2, 4):
            emit_ln_alloc(c_pre)

        acc = singles.tile([DHV, H, NQ], F32)
        nc.vector.memset(acc, 0.0)
        eps_t = singles.tile([128, 1], F32)
        nc.vector.memset(eps_t, EPS)
        qT = singles.tile([DHA, H, NQ], F16)
        pooledT = singles.tile([DHA, H, NQ], F16)

        # ---- query path. Emitted INSIDE loop 1 at h==1, so K(0)/K(1) lead
        # the PE FIFO instead of q-transposes stalled on the q-LN chain.
        zqT = singles.tile([128, DM // 128, NQ], F16)

        def emit_q_ln():
          for qt in range(2):
            qtile = qtiles[:, qt, :]
            st = work.tile([128, 2, 6], F32, tag="qstats")
            qv = qtile.rearrange("p (s d) -> p s d", s=2)
            for s in range(2):
                nc.vector.bn_stats(out=st[:, s, :], in_=qv[:, s, :])
            mv = work.tile([128, 2], F32, tag="qmv")
            nc.vector.bn_aggr(out=mv, in_=st)
            qrstd = work.tile([128, 1], F32, tag="qrstd")
            emit_rsqrt(nc, work, qrstd, mv[:, 1:2], EPS, tag="qr")
            zq = work.tile([128, DM], F16, tag="zq")
            nc.vector.tensor_scalar(
                out=zq, in0=qtile,
                scalar1=mv[:, 0:1], scalar2=qrstd,
                op0=mybir.AluOpType.subtract, op1=mybir.AluOpType.mult,
            )
            pt1 = pz_pool.tile([128, 4, 128], F16, tag="pz")
            for i in range(4):
                nc.tensor.transpose(
                    pt1[:, i, :], zq[:, i * 128 : (i + 1) * 128], identh
                )
            nc.vector.tensor_copy(
                out=zqT[:, 0:4, qt * 128 : (qt + 1) * 128], in_=pt1
            )
            pt2 = pz_pool.tile([128, 4, 128], F16, tag="pz")
            for i in range(2):
                nc.tensor.transpose(
                    pt2[:, i, :], zq[:, (4 + i) * 128 : (5 + i) * 128], identh
                )
            nc.vector.tensor_copy(
                out=zqT[:, 4:6, qt * 128 : (qt + 1) * 128], in_=pt2[:, 0:2, :]
            )
        def emit_q_proj():
            for h in range(H):
                pq = ps_pool.tile([128, NQ], F32, tag="ps")
                for cc in range(DM // 128):
                    nc.tensor.matmul(
                        pq[0:DHA, :],
                        wq_sb[:, cc, h * DHA : (h + 1) * DHA],
                        zqT[:, cc, :],
                        start=(cc == 0), stop=(cc == DM // 128 - 1),
                    )
                nc.vector.tensor_scalar_add(
                    out=qT[:, h, :], in0=pq[0:DHA, :], scalar1=qb_sb[:, h : h + 1]
                )

        # ---- software-pipelined main loop ----
        def emit_ln_stats(c, rt):
            xts, mv_all, _ = ln_state[c]
            st = work.tile([128, 2, 6], F32, tag="xstats")
            xv = xts[:, rt, :].rearrange("p (s d) -> p s d", s=2)
            for s in range(2):
                nc.vector.bn_stats(out=st[:, s, :], in_=xv[:, s, :])
            nc.vector.bn_aggr(out=mv_all[:, rt, :], in_=st)

        def emit_ln_rstd(c):
            _, mv_all, rstd4 = ln_state[c]
            emit_rsqrt(nc, work, rstd4, mv_all[:, :, 1], EPS, tag="xr")

        def emit_v_proj(c):
            zT = zTs[c]
            _, mv_all, rstd4 = ln_state[c]
            v_aug = chunkp.tile([128, RT, H, DHV], F16, tag="vaug")
            nc.vector.memset(v_aug[:, :, :, DH + 1 : DH + 2], 1.0)
            for rt in range(RT):
                # col 96 = r_j*mu_j (rides the attention matmul to build the
                # -mu*csv correction); built from the ones col just memset
                nc.vector.tensor_scalar(
                    out=v_aug[:, rt, :, DH : DH + 1],
                    in0=v_aug[:, rt, :, DH + 1 : DH + 2],
                    scalar1=mv_all[:, rt, 0:1], scalar2=rstd4[:, rt : rt + 1],
                    op0=mybir.AluOpType.mult, op1=mybir.AluOpType.mult,
                )
            for rt in range(RT):
                for vh in range(2):
                    pv = pv_pool.tile([128, 384], F32, tag="pv")
                    j0 = WKA + vh * 384
                    for cc in range(CC):
                        nc.tensor.matmul(
                            pv, zT[:, cc, rt * 128 : (rt + 1) * 128],
                            wkv_sb[:, cc, j0 : j0 + 384],
                            start=(cc == 0), stop=(cc == CC - 1),
                        )
                    # drain applies the per-token LN scale r_j. On the DVE:
                    # scale-AP activations cost +51-91ns each on the scalar
                    # engine, and scalar (which also runs the ps-ring-gating
                    # Exps) is the busier engine by far.
                    nc.vector.tensor_scalar(
                        out=v_aug[:, rt, vh * 4 : (vh + 1) * 4, 0:DH],
                        in0=pv.rearrange("p (h d) -> p h d", d=DH),
                        scalar1=rstd4[:, rt : rt + 1], scalar2=None,
                        op0=mybir.AluOpType.mult,
                    )
            vaugs[c] = v_aug

        def emit_k_head(c, h):
            zT = zTs[c]
            pk = pk_pool.tile([DHA, CHUNK], F32, tag="pk")
            for cc in range(CC):
                nc.tensor.matmul(
                    pk, wkv_sb[:, cc, h * DHA : (h + 1) * DHA], zT[:, cc, :],
                    start=(cc == 0), stop=(cc == CC - 1),
                )
            return pk

        def emit_k_drain(pk):
            # scalar-engine drain, emitted AFTER the first exp pair so Exps
            # lead the scalar queue; row 96 is -mu_j straight from the PE
            kT = attp.tile([DHA, CHUNK], F16, tag="kT")
            # vector-engine drain: keeps the scalar engine free for the Exps
            # that gate the sim psum ring
            nc.vector.tensor_copy(out=kT, in_=pk)
            return kT

        def emit_sim_pair(c, h, kT, at4, rtp):
            rstd4 = ln_state[c][2]
            for i in range(2):
                rt = rtp * 2 + i
                ps = ps_pool.tile([128, NQ], F32, tag="ps")
                nc.tensor.matmul(
                    ps, kT[:, rt * 128 : (rt + 1) * 128], qT[:, h, :],
                    start=True, stop=True,
                )
                # psum holds q.kraw - mu*t; Exp's per-partition scale r_j
                # completes the LayerNorm fold
                nc.scalar.activation(
                    out=at4[:, rt, :], in_=ps,
                    func=mybir.ActivationFunctionType.Exp,
                    scale=rstd4[:, rt : rt + 1],
                )

        def emit_pv_head(c, h, at4):
            v_aug = vaugs[c]
            pacc = pa_pool.tile([DHV, NQ], F32, tag="pa")
            for rt in range(RT):
                nc.tensor.matmul(
                    pacc, v_aug[:, rt, h, :], at4[:, rt, :],
                    start=(rt == 0), stop=(rt == RT - 1),
                )
            nc.vector.tensor_add(out=acc[:, h, :], in0=acc[:, h, :], in1=pacc)

        def emit_finale_head(h):
            # acc[:, h, :] -> per-query normalize -> pooledT[:, h, :]
            # col 96 of pN is s' = (sum a*rmu)/denom, consumed by W_out row 96
            for qh in range(2):
                ps = ps_pool.tile([128, NQ], F32, tag="ps")
                nc.tensor.transpose(
                    ps[:, 0:DHV],
                    acc[:, h, qh * 128 : (qh + 1) * 128],
                    identf[0:DHV, 0:DHV],
                )
                rz = work.tile([128, 1], F32, tag="rz")
                nc.vector.reciprocal(out=rz, in_=ps[:, DH + 1 : DH + 2])
                pN = work.tile([128, DHA], F16, tag="pN")
                nc.vector.tensor_scalar(
                    out=pN, in0=ps[:, 0:DHA], scalar1=rz, scalar2=None,
                    op0=mybir.AluOpType.mult,
                )
                pt = pz_pool.tile([128, 4, 128], F16, tag="pz")
                nc.tensor.transpose(pt[0:DHA, 0, :], pN, identh)
                nc.vector.tensor_copy(
                    out=pooledT[:, h, qh * 128 : (qh + 1) * 128],
                    in_=pt[0:DHA, 0, :],
                )

        for ch in range(N_CHUNKS + 1):
            if ch == 0:
                # chunk-0 transposes on the PE: useful work for the
                # otherwise idle DMA-preamble window (x0-nat lands ~10us,
                # wkv-K ~20us)
                xts0_t = ln_state[0][0]
                zT0 = zTs[0]
                for rt in range(RT):
                    for ccp in range(2):
                        pt = pz_pool.tile([128, 4, 128], F16, tag="pz")
                        for i in range(4):
                            cc = ccp * 4 + i
                            nc.tensor.transpose(
                                pt[:, i, :],
                                xts0_t[:, rt, cc * 128 : (cc + 1) * 128],
                                identh,
                            )
                        nc.vector.tensor_copy(
                            out=zT0[:, ccp * 4 : (ccp + 1) * 4,
                                    rt * 128 : (rt + 1) * 128],
                            in_=pt,
                        )
                for c0 in (0, 1):
                    for rt in range(RT):
                        emit_ln_stats(c0, rt)
                    emit_ln_rstd(c0)
                continue
            c = ch - 1
            # V projection first so its PSUM drains lead the scalar queue
            # (they gate attn@v). EXCEPT loop 1: wkv-V is still streaming,
            # so loop 1 runs K-first, V mid-loop, attn@v after the loop.
            defer_v = ch == 1
            if not defer_v:
                emit_v_proj(c)
            if ch >= 2 and ch + 2 < N_CHUNKS:
                emit_ln_alloc(ch + 2)
            if ch == 5:
                nc.sync.dma_start(
                    out=wout_sb, in_=wout.rearrange("(h p) j -> p h j", p=DHA)
                )
            # stats drip: h 3..7 never head-blocks the vector queue on a
            # pending x DMA (x(ch+1) landed a loop ago); loop 1's drip moves
            # to the tail, after the deferred attn@v
            drip0 = 99 if ch == 1 else 3
            kTs, pks, at4s = {}, {}, {}
            for h in range(H + 2):
                # K(h) first: gives kT(h-1)'s drain a full PE-burst of slack.
                # sims for h-1 split around pv(h-2) so the psum slots
                # recycle only after the matching Exp has drained them.
                if h < H:
                    pks[h] = emit_k_head(c, h)
                if defer_v and h == 1:
                    # q path lands here: K(0)+K(1) lead the PE FIFO while
                    # the q-LN chain and wq DMA finish; sims need qT from
                    # h==1 onward
                    emit_q_ln()
                    emit_q_proj()
                if 1 <= h <= H:
                    kT_prev = kTs.pop(h - 1)
                    at4 = attp.tile([128, RT, NQ], F16, tag="at")
                    at4s[h - 1] = at4
                    emit_sim_pair(c, h - 1, kT_prev, at4, 0)
                if h < H:
                    kTs[h] = emit_k_drain(pks.pop(h))
                if h >= 2 and not defer_v:
                    emit_pv_head(c, h - 2, at4s.pop(h - 2))
                    if ch == N_CHUNKS:
                        emit_finale_head(h - 2)
                if 1 <= h <= H:
                    emit_sim_pair(c, h - 1, kT_prev, at4s[h - 1], 1)
                if defer_v and h == 4:
                    emit_v_proj(c)
                if ch + 1 < N_CHUNKS:
                    if drip0 <= h < drip0 + RT:
                        emit_ln_stats(ch + 1, h - drip0)
                    if h == drip0 + RT and drip0 + RT <= H + 1:
                        emit_ln_rstd(ch + 1)
            if defer_v:
                for h2 in range(H):
                    emit_pv_head(c, h2, at4s.pop(h2))
                if ch + 1 < N_CHUNKS:
                    for rt in range(RT):
                        emit_ln_stats(ch + 1, rt)
                    emit_ln_rstd(ch + 1)

        # ---- output projection ----
        ob_bc = None
        if has_out_bias:
            ob = singles.tile([1, DM], F32)
            nc.sync.dma_start(out=ob, in_=out_bias[:, :])
            ob_bc = singles.tile([128, DM], F32)
            nc.gpsimd.partition_broadcast(ob_bc, ob)
        for qh in range(2):
            pja = pv_pool.tile([128, 384], F32, tag="pv")
            pjb = pv_pool.tile([128, 384], F32, tag="pv")
            pj = [pja, pjb]
            for h in range(H):
                for j in range(2):
                    nc.tensor.matmul(
                        pj[j], pooledT[:, h, qh * 128 : (qh + 1) * 128],
                        wout_sb[:, h, j * 384 : (j + 1) * 384],
                        start=(h == 0), stop=(h == H - 1),
                    )
            ot = work.tile([128, DM], F32, tag="ot")
            for j in range(2):
                sl = slice(j * 384, (j + 1) * 384)
                if ob_bc is not None:
                    nc.vector.tensor_add(out=ot[:, sl], in0=pj[j], in1=ob_bc[:, sl])
                else:
                    nc.vector.tensor_copy(out=ot[:, sl], in_=pj[j])
            nc.sync.dma_start(out=out[qh * 128 : (qh + 1) * 128, :], in_=ot)
    nc.compile()
    return nc


_NC_CACHE = {}
_TRACE = False


def kernel(**inputs):
    x = np.asarray(inputs["x"], dtype=np.float32)
    query = np.asarray(inputs["query"], dtype=np.float32)
    ln_k_g = np.asarray(inputs["ln_k_g"], dtype=np.float32)
    ln_k_b = np.asarray(inputs["ln_k_b"], dtype=np.float32)
    ln_q_g = np.asarray(inputs["ln_q_g"], dtype=np.float32)
    ln_q_b = np.asarray(inputs["ln_q_b"], dtype=np.float32)
    W_q = np.asarray(inputs["W_q"], dtype=np.float32)
    W_kv = np.asarray(inputs["W_kv"], dtype=np.float32)
    W_out = np.asarray(inputs["W_out"], dtype=np.float32)

    scale = DH ** -0.5
    wkv_f = ln_k_g[:, None] * W_kv  # [1024, 1536]
    wk = wkv_f[:, :DM]
    wv = wkv_f[:, DM:]
    cs_k = wk.sum(axis=0)  # [768]
    cs_v = wv.sum(axis=0)  # [768]

    # K weights augmented with a constant -1/CTX column per head: the K
    # projection psum row 96 becomes -mu_j exactly.
    wk_aug = np.empty((CTX, H, DHA), np.float32)
    for h in range(H):
        wk_aug[:, h, :DH] = wk[:, h * DH : (h + 1) * DH]
        wk_aug[:, h, DH] = -1.0 / CTX
    wkvg = np.concatenate(
        [wk_aug.reshape(CTX, WKA), wv], axis=1
    ).astype(np.float16)  # [1024, 776+768]

    wqg_f = ln_q_g[:, None] * W_q * scale
    qbias_f = (ln_q_b @ W_q) * scale  # [768]
    # extra W_q output col per head: t_i = q_i . cs_k (for the -mu*t term)
    wq_aug = np.empty((DM, H, DHA), np.float32)
    qb_aug = np.empty((H, DHA), np.float32)
    for h in range(H):
        hs = slice(h * DH, (h + 1) * DH)
        wq_aug[:, h, :DH] = wqg_f[:, hs]
        wq_aug[:, h, DH] = wqg_f[:, hs] @ cs_k[hs]
        qb_aug[h, :DH] = qbias_f[hs]
        qb_aug[h, DH] = qbias_f[hs] @ cs_k[hs]
    wqg = wq_aug.reshape(DM, WKA).astype(np.float16)
    qbias = qb_aug.reshape(WKA).astype(np.float32)

    # W_out augmented with row 96 per head: -(cs_v_h @ W_out_h) cancels the
    # -mu*cs_v part of the V fold (multiplied by s' from pooledT row 96).
    wout_aug = np.empty((H, DHA, DM), np.float32)
    for h in range(H):
        hs = slice(h * DH, (h + 1) * DH)
        wout_aug[h, :DH, :] = W_out[hs, :]
        wout_aug[h, DH, :] = -(cs_v[hs] @ W_out[hs, :])
    wout = wout_aug.reshape(WKA, DM).astype(np.float16)

    kv_bias = ln_k_b @ W_kv
    vb = kv_bias[DM:]
    has_out_bias = bool(np.any(vb != 0.0))
    x16 = x.astype(np.float16)

    key = has_out_bias
    if key not in _NC_CACHE:
        _NC_CACHE[key] = build_nc(has_out_bias)
    nc = _NC_CACHE[key]

    shared = dict(
        query=query.astype(np.float16), wkvg=wkvg, wqg=wqg,
        wout=wout,
        qbias=qbias,
        identh=np.eye(128, dtype=np.float16),
        identf=np.eye(128, dtype=np.float32),
    )
    if has_out_bias:
        shared["out_bias"] = (vb @ W_out).astype(np.float32).reshape(1, DM)
    in_maps = [dict(x=x16[i], **shared) for i in range(N_CORES)]
    res = run_bass_kernel_spmd(
        nc, in_maps, core_ids=list(range(N_CORES)), trace=_TRACE
    )
    kernel.last_result = res
    out = np.stack([np.asarray(res.results[i]["out"]) for i in range(N_CORES)])
    return out.astype(np.float32)


if __name__ == "__main__":
    rng = np.random.default_rng(0)
    ins = {
        "x": rng.standard_normal((B, N, CTX), dtype=np.float32),
        "query": rng.standard_normal((NQ, DM), dtype=np.float32),
        "ln_k_g": np.ones(CTX, np.float32),
        "ln_k_b": np.zeros(CTX, np.float32),
        "ln_q_g": np.ones(DM, np.float32),
        "ln_q_b": np.zeros(DM, np.float32),
        "W_q": rng.standard_normal((DM, DM), dtype=np.float32) * DM ** -0.5,
        "W_kv": rng.standard_normal((CTX, 2 * DM), dtype=np.float32) * CTX ** -0.5,
        "W_out": rng.standard_normal((DM, DM), dtype=np.float32) * DM ** -0.5,
    }
    o = kernel(**ins)
    print("out", o.shape, o.dtype, float(np.abs(o).mean()))


# revision 71
# speedup vs baseline: 1.2756x; 1.2756x over previous
"""AttentionalPooler Trainium2 kernel (fp16 PE paths).

Data-parallel over batch: each of 8 NeuronCores processes one batch element
(x[i]: [4096, 1024], shipped to the device pre-cast to fp16).

Key design points vs the f32r baseline (447us -> ~340us):
  - All matmul operands fp16: streams run 1 row/cycle (f32r ran ~2.6x slower
    with fp32_mode=HIGH double passes) and LDWEIGHTS is ~2x faster (+FWL).
  - LN gain and the dh^-0.5 scale are folded into the weights on the host;
    the K-path bias is dropped entirely (constant per query -> softmax
    invariant); a nonzero V bias is folded into an output-side bias add.
  - rsqrt(var+eps) computed on the vector engine via Quake-style Newton
    iteration -> the scalar engine only ever runs EXP (no ACT table thrash).
  - K projection streams N=512 per weight load (chunk = 512 rows); V
    projection keeps the transposed activations stationary and streams the
    full 768 W_v columns (100% PE width).
  - Software-pipelined chunks: head h's K matmuls cover head h-1's sim psum
    drains and Exp latency; the NEXT chunk's LayerNorm (vector) and PE
    transposes are dripped into the head loop so they neither stall the PE
    nor delay this chunk's PSUM drains in the vector queue.
  - Softmax denominator via an appended ones-column in v_aug; the [97, 256]
    per-head accumulators are transposed at the end so the normalization is a
    per-partition (per-query) tensor_scalar; the qh=0 half of the output
    projection accumulates inline as heads finalize.
"""

import sys

for p in ("/opt/trn_rl_repo",):
    if p not in sys.path:
        sys.path.insert(0, p)

import numpy as np

import concourse.bass as bass
import concourse.tile as tile
from concourse import bacc
from concourse import mybir
from concourse.bass_utils import run_bass_kernel_spmd

F32 = mybir.dt.float32
F16 = mybir.dt.float16
I32 = mybir.dt.int32

N_CORES = 8
B, N, CTX = 8, 4096, 1024
NQ, DM, H = 256, 768, 8
DH = DM // H  # 96
EPS = 1e-5
CHUNK = 512
N_CHUNKS = N // CHUNK  # 8
RT = CHUNK // 128  # 4 row-tiles per chunk
CC = CTX // 128  # 8 contraction chunks
RSQRT_MAGIC = 0x5F3759DF


def emit_rsqrt(nc, pool, out, var, eps, tag):
    """out = 1/sqrt(var + eps), [128, n] f32, vector engine only."""
    n = out.shape[1]
    ve = pool.tile([128, n], F32, tag=f"{tag}_ve")
    nc.vector.tensor_scalar_add(out=ve, in0=var, scalar1=float(eps))
    sh = pool.tile([128, n], I32, tag=f"{tag}_sh")
    nc.vector.tensor_scalar(
        out=sh, in0=ve.bitcast(I32), scalar1=1, scalar2=None,
        op0=mybir.AluOpType.logical_shift_right,
    )
    magic = pool.tile([128, n], I32, tag=f"{tag}_mg")
    nc.vector.memset(magic, RSQRT_MAGIC)
    # y0 = bitcast(magic - (i >> 1))
    nc.vector.scalar_tensor_tensor(
        out=out.bitcast(I32), in0=magic, scalar=0, in1=sh,
        op0=mybir.AluOpType.bypass, op1=mybir.AluOpType.subtract,
    )
    t = pool.tile([128, n], F32, tag=f"{tag}_t")
    for _ in range(2):  # two Newton steps: rel err ~4e-6
        nc.vector.tensor_mul(out=t, in0=out, in1=out)
        nc.vector.tensor_mul(out=t, in0=t, in1=ve)
        nc.vector.tensor_scalar(
            out=t, in0=t, scalar1=-0.5, scalar2=1.5,
            op0=mybir.AluOpType.mult, op1=mybir.AluOpType.add,
        )
        nc.vector.tensor_mul(out=out, in0=out, in1=t)


def build_nc(has_out_bias):
    nc = bacc.Bacc("TRN2", debug=False)
    x = nc.dram_tensor("x", [N, CTX], F16, kind="ExternalInput")
    query = nc.dram_tensor("query", [NQ, DM], F16, kind="ExternalInput")
    wkvg = nc.dram_tensor("wkvg", [CTX, 2 * DM], F16, kind="ExternalInput")
    wqg = nc.dram_tensor("wqg", [DM, DM], F16, kind="ExternalInput")
    wout = nc.dram_tensor("wout", [DM, DM], F16, kind="ExternalInput")
    qbias = nc.dram_tensor("qbias", [DM], F32, kind="ExternalInput")
    identh_p = nc.dram_tensor("identh", [128, 128], F16, kind="ExternalInput")
    identf_p = nc.dram_tensor("identf", [128, 128], F32, kind="ExternalInput")
    if has_out_bias:
        out_bias = nc.dram_tensor("out_bias", [1, DM], F32, kind="ExternalInput")
    out = nc.dram_tensor("out", [NQ, DM], F32, kind="ExternalOutput")

    from contextlib import ExitStack

    with tile.TileContext(nc) as tc, ExitStack() as es:
        singles = es.enter_context(tc.tile_pool(name="singles", bufs=1))
        work = es.enter_context(tc.tile_pool(name="work", bufs=3))
        xpool = es.enter_context(tc.tile_pool(name="xpool", bufs=12))
        chunkp = es.enter_context(tc.tile_pool(name="chunkp", bufs=2))
        attp = es.enter_context(tc.tile_pool(name="attp", bufs=4))
        # PSUM pools: bank-granular; 1+1+2+3+1 = 8 banks exactly
        pz_pool = es.enter_context(tc.tile_pool(name="pz", bufs=1, space="PSUM"))
        pk_pool = es.enter_context(tc.tile_pool(name="pk", bufs=2, space="PSUM"))
        pv_pool = es.enter_context(tc.tile_pool(name="pv", bufs=2, space="PSUM"))
        ps_pool = es.enter_context(tc.tile_pool(name="ps", bufs=2, space="PSUM"))
        pa_pool = es.enter_context(tc.tile_pool(name="pa", bufs=1, space="PSUM"))

        # ---- DMAs: x + wkv on the SP queue, q-path on the ACT queue (parallel)
        # Critical-path balance across the two HWDGE queues:
        #   ACT:  qtiles -> wkv-V (gates V(0)) -> wq -> rest
        #   SYNC: x0 (gates chunk-0 LN/transposes) -> wkv-K -> x1
        identh = singles.tile([128, 128], F16)
        nc.scalar.dma_start(out=identh, in_=identh_p[:, :])
        qtiles = singles.tile([128, 2, DM], F16)
        nc.scalar.dma_start(out=qtiles[:, 0, :], in_=query[0:128, :])
        nc.scalar.dma_start(out=qtiles[:, 1, :], in_=query[128:256, :])
        xts0 = []
        for rt in range(RT):
            xt = xpool.tile([128, CTX], F16, tag="xt")
            nc.sync.dma_start(out=xt, in_=x[rt * 128 : (rt + 1) * 128, :])
            xts0.append(xt)
        wkv_sb = singles.tile([128, CC, 2 * DM], F16)
        wkv_re = wkvg.rearrange("(cc p) j -> p cc j", p=128)
        nc.scalar.dma_start(out=wkv_sb[:, :, DM:], in_=wkv_re[:, :, DM:])
        nc.sync.dma_start(out=wkv_sb[:, :, 0:DM], in_=wkv_re[:, :, 0:DM])
        wq_sb = singles.tile([128, DM // 128, DM], F16)
        nc.scalar.dma_start(out=wq_sb, in_=wqg.rearrange("(cc p) j -> p cc j", p=128))
        qb_sb = singles.tile([DH, H], F32)
        nc.scalar.dma_start(out=qb_sb, in_=qbias.rearrange("(h p) -> p h", p=DH))
        identf = singles.tile([128, 128], F32)
        nc.scalar.dma_start(out=identf, in_=identf_p[:, :])
        xts1 = []
        for rt in range(RT):
            xt = xpool.tile([128, CTX], F16, tag="xt")
            nc.sync.dma_start(out=xt, in_=x[CHUNK + rt * 128 : CHUNK + (rt + 1) * 128, :])
            xts1.append(xt)
        wout_sb = singles.tile([DH, H, DM], F16)

        acc = singles.tile([DH + 1, H, NQ], F32)
        nc.vector.memset(acc, 0.0)
        eps_t = singles.tile([128, 1], F32)
        nc.vector.memset(eps_t, EPS)
        qT = singles.tile([DH, H, NQ], F16)
        pooledT = singles.tile([DH, H, NQ], F16)

        # ---- query path (PE work while x/wkv DMAs stream in) ----
        zqT = singles.tile([128, DM // 128, NQ], F16)
        for qt in range(2):
            qtile = qtiles[:, qt, :]
            st = work.tile([128, 2, 6], F32, tag="qstats")
            qv = qtile.rearrange("p (s d) -> p s d", s=2)
            for s in range(2):
                nc.vector.bn_stats(out=st[:, s, :], in_=qv[:, s, :])
            mv = work.tile([128, 2], F32, tag="qmv")
            nc.vector.bn_aggr(out=mv, in_=st)
            qrstd = work.tile([128, 1], F32, tag="qrstd")
            emit_rsqrt(nc, work, qrstd, mv[:, 1:2], EPS, tag="qr")
            zq = work.tile([128, DM], F16, tag="zq")
            nc.vector.tensor_scalar(
                out=zq, in0=qtile,
                scalar1=mv[:, 0:1], scalar2=qrstd,
                op0=mybir.AluOpType.subtract, op1=mybir.AluOpType.mult,
            )
            pt1 = pz_pool.tile([128, 4, 128], F16, tag="pz")
            for i in range(4):
                nc.tensor.transpose(
                    pt1[:, i, :], zq[:, i * 128 : (i + 1) * 128], identh
                )
            nc.vector.tensor_copy(
                out=zqT[:, 0:4, qt * 128 : (qt + 1) * 128], in_=pt1
            )
            pt2 = pz_pool.tile([128, 4, 128], F16, tag="pz")
            for i in range(2):
                nc.tensor.transpose(
                    pt2[:, i, :], zq[:, (4 + i) * 128 : (5 + i) * 128], identh
                )
            nc.vector.tensor_copy(
                out=zqT[:, 4:6, qt * 128 : (qt + 1) * 128], in_=pt2[:, 0:2, :]
            )
        for h in range(H):
            pq = ps_pool.tile([128, NQ], F32, tag="ps")
            for cc in range(DM // 128):
                nc.tensor.matmul(
                    pq[0:DH, :],
                    wq_sb[:, cc, h * DH : (h + 1) * DH],
                    zqT[:, cc, :],
                    start=(cc == 0), stop=(cc == DM // 128 - 1),
                )
            nc.vector.tensor_scalar_add(
                out=qT[:, h, :], in0=pq[0:DH, :], scalar1=qb_sb[:, h : h + 1]
            )

        # ---- software-pipelined main loop ----
        zrows, rstds, zTs, vaugs = {}, {}, {}, {}
        ln_state = {}
        xq = {}

        def emit_x_dma(c):
            xts = []
            for rt in range(RT):
                xt = xpool.tile([128, CTX], F16, tag="xt")
                r0 = c * CHUNK + rt * 128
                nc.sync.dma_start(out=xt, in_=x[r0 : r0 + 128, :])
                xts.append(xt)
            return xts

        def emit_ln_alloc(c, xts=None, fast_rstd=False):
            if xts is None:
                xts = emit_x_dma(c)
            mv_all = chunkp.tile([128, RT, 2], F32, tag="mv")
            zrow4 = chunkp.tile([128, RT, CTX], F16, tag="zrow")
            rstd4 = chunkp.tile([128, RT], F32, tag="rstd")
            zT_c = chunkp.tile([128, CC, CHUNK], F16, tag="zT")
            ln_state[c] = (xts, mv_all, rstd4, fast_rstd)
            zrows[c] = zrow4
            zTs[c] = zT_c

        def emit_ln_stats(c, rt):
            xts, mv_all, _, _ = ln_state[c]
            st = work.tile([128, 2, 6], F32, tag="xstats")
            xv = xts[rt].rearrange("p (s d) -> p s d", s=2)
            for s in range(2):
                nc.vector.bn_stats(out=st[:, s, :], in_=xv[:, s, :])
            nc.vector.bn_aggr(out=mv_all[:, rt, :], in_=st)

        def emit_ln_rstd(c):
            _, mv_all, rstd4, fast = ln_state[c]
            if fast:
                xstd = work.tile([128, RT], F32, tag="xstd")
                nc.scalar.activation(
                    out=xstd, in_=mv_all[:, :, 1],
                    func=mybir.ActivationFunctionType.Sqrt, bias=eps_t,
                )
                nc.vector.reciprocal(out=rstd4, in_=xstd)
            else:
                emit_rsqrt(nc, work, rstd4, mv_all[:, :, 1], EPS, tag="xr")

        def emit_ln_zrow(c, rt):
            xts, mv_all, rstd4, _ = ln_state[c]
            nc.vector.tensor_scalar(
                out=zrows[c][:, rt, :], in0=xts[rt],
                scalar1=mv_all[:, rt, 0:1], scalar2=rstd4[:, rt : rt + 1],
                op0=mybir.AluOpType.subtract, op1=mybir.AluOpType.mult,
            )

        def emit_load_ln(c, xts=None, fast_rstd=False):
            emit_ln_alloc(c, xts=xts, fast_rstd=fast_rstd)
            for rt in range(RT):
                emit_ln_stats(c, rt)
            emit_ln_rstd(c)
            for rt in range(RT):
                emit_ln_zrow(c, rt)

        def transpose_slices(c):
            """8 closures, each: 4 transposes + 1 drain into zT(c)."""
            zrow4, zT = zrows[c], zTs[c]
            out = []
            for rt in range(RT):
                for ccp in range(2):
                    def emit(rt=rt, ccp=ccp):
                        pt = pz_pool.tile([128, 4, 128], F16, tag="pz")
                        for i in range(4):
                            cc = ccp * 4 + i
                            nc.tensor.transpose(
                                pt[:, i, :],
                                zrow4[:, rt, cc * 128 : (cc + 1) * 128],
                                identh,
                            )
                        nc.vector.tensor_copy(
                            out=zT[:, ccp * 4 : (ccp + 1) * 4,
                                   rt * 128 : (rt + 1) * 128],
                            in_=pt,
                        )
                    out.append(emit)
            return out

        def emit_v_proj(c):
            zT = zTs[c]
            v_aug = chunkp.tile([128, RT, H, DH + 1], F16, tag="vaug")
            for rt in range(RT):
                for vh in range(2):
                    pv = pv_pool.tile([128, 384], F32, tag="pv")
                    j0 = DM + vh * 384
                    for cc in range(CC):
                        nc.tensor.matmul(
                            pv, zT[:, cc, rt * 128 : (rt + 1) * 128],
                            wkv_sb[:, cc, j0 : j0 + 384],
                            start=(cc == 0), stop=(cc == CC - 1),
                        )
                    # scalar engine: idle during the V phase, closer to PSUM
                    nc.scalar.copy(
                        out=v_aug[:, rt, vh * 4 : (vh + 1) * 4, 0:DH],
                        in_=pv.rearrange("p (h d) -> p h d", d=DH),
                    )
            nc.vector.memset(v_aug[:, :, :, DH : DH + 1], 1.0)
            vaugs[c] = v_aug

        def emit_k_head(c, h):
            zT = zTs[c]
            pk = pk_pool.tile([DH, CHUNK], F32, tag="pk")
            for cc in range(CC):
                nc.tensor.matmul(
                    pk, wkv_sb[:, cc, h * DH : (h + 1) * DH], zT[:, cc, :],
                    start=(cc == 0), stop=(cc == CC - 1),
                )
            return pk

        def emit_k_drain(pk):
            # scalar-engine drain, emitted AFTER the first exp pair so Exps
            # lead the scalar queue (they free the sim PSUM slots); pk bufs=2
            # tolerates the drain landing late
            kT = attp.tile([DH, CHUNK], F16, tag="kT")
            nc.scalar.copy(out=kT, in_=pk)
            return kT

        def emit_sim_pair(c, h, kT, at4, rtp):
            for i in range(2):
                rt = rtp * 2 + i
                ps = ps_pool.tile([128, NQ], F32, tag="ps")
                nc.tensor.matmul(
                    ps, kT[:, rt * 128 : (rt + 1) * 128], qT[:, h, :],
                    start=True, stop=True,
                )
                nc.scalar.activation(
                    out=at4[:, rt, :], in_=ps,
                    func=mybir.ActivationFunctionType.Exp,
                )

        def emit_pv_head(c, h, at4):
            v_aug = vaugs[c]
            pacc = pa_pool.tile([DH + 1, NQ], F32, tag="pa")
            for rt in range(RT):
                nc.tensor.matmul(
                    pacc, v_aug[:, rt, h, :], at4[:, rt, :],
                    start=(rt == 0), stop=(rt == RT - 1),
                )
            nc.vector.tensor_add(out=acc[:, h, :], in0=acc[:, h, :], in1=pacc)

        def emit_finale_head(h):
            # acc[:, h, :] -> per-query normalize -> pooledT[:, h, :]
            for qh in range(2):
                ps = ps_pool.tile([128, NQ], F32, tag="ps")
                nc.tensor.transpose(
                    ps[:, 0 : DH + 1],
                    acc[:, h, qh * 128 : (qh + 1) * 128],
                    identf[0 : DH + 1, 0 : DH + 1],
                )
                rz = work.tile([128, 1], F32, tag="rz")
                nc.vector.reciprocal(out=rz, in_=ps[:, DH : DH + 1])
                pN = work.tile([128, DH], F16, tag="pN")
                nc.vector.tensor_scalar(
                    out=pN, in0=ps[:, 0:DH], scalar1=rz, scalar2=None,
                    op0=mybir.AluOpType.mult,
                )
                pt = pz_pool.tile([128, 4, 128], F16, tag="pz")
                nc.tensor.transpose(pt[0:DH, 0, :], pN, identh)
                nc.vector.tensor_copy(
                    out=pooledT[:, h, qh * 128 : (qh + 1) * 128],
                    in_=pt[0:DH, 0, :],
                )

        for ch in range(N_CHUNKS + 1):
            if ch == 0:
                # LN for chunks 0 AND 1 up front: the LN pipeline runs one
                # full iteration ahead of the transposes that consume it
                emit_load_ln(0, xts=xts0)
                for emit in transpose_slices(0):
                    emit()
                emit_load_ln(1, xts=xts1)
                xq[2] = emit_x_dma(2)
                continue
            c = ch - 1
            # V projection first so its PSUM drains lead the vector queue
            # (they gate attn@v). Chunk ch+1's LN is dripped into the head
            # loop (its zrow is consumed only next iteration).
            emit_v_proj(c)
            if ch + 2 < N_CHUNKS:
                xq[ch + 2] = emit_x_dma(ch + 2)
            if ch + 1 < N_CHUNKS:
                emit_ln_alloc(ch + 1, xts=xq.pop(ch + 1))
            if ch == 2:
                nc.scalar.dma_start(
                    out=wout_sb, in_=wout.rearrange("(h p) j -> p h j", p=DH)
                )
            filler = transpose_slices(ch) if ch < N_CHUNKS else None
            fi = 0
            kTs, pks, at4s = {}, {}, {}
            for h in range(H + 2):
                # K(h) first: gives kT(h-1)'s drain a full PE-burst of slack.
                # sims for h-1 split around pv(h-2) so the psum slots
                # recycle only after the matching Exp has drained them.
                if h < H:
                    pks[h] = emit_k_head(c, h)
                if 1 <= h <= H:
                    kT_prev = kTs.pop(h - 1)
                    at4 = attp.tile([128, RT, NQ], F16, tag="at")
                    at4s[h - 1] = at4
                    emit_sim_pair(c, h - 1, kT_prev, at4, 0)
                if h < H:
                    kTs[h] = emit_k_drain(pks.pop(h))
                if h >= 2:
                    emit_pv_head(c, h - 2, at4s.pop(h - 2))
                    if filler is None:
                        emit_finale_head(h - 2)
                if 1 <= h <= H:
                    emit_sim_pair(c, h - 1, kT_prev, at4s[h - 1], 1)
                # this chunk's transposes early (zrow ready since last iter);
                # NEXT chunk's LN vector work dripped across the head loop
                if filler is not None and 1 <= h <= 4:
                    for _ in range(2):
                        if fi < len(filler):
                            filler[fi](); fi += 1
                if ch + 1 < N_CHUNKS:
                    if h < RT:
                        emit_ln_stats(ch + 1, h)
                    if h == RT:
                        emit_ln_rstd(ch + 1)
                    if RT <= h < 2 * RT:
                        emit_ln_zrow(ch + 1, h - RT)
            if filler is not None:
                while fi < len(filler):
                    filler[fi](); fi += 1

        # ---- output projection ----
        ob_bc = None
        if has_out_bias:
            ob = singles.tile([1, DM], F32)
            nc.sync.dma_start(out=ob, in_=out_bias[:, :])
            ob_bc = singles.tile([128, DM], F32)
            nc.gpsimd.partition_broadcast(ob_bc, ob)
        for qh in range(2):
            pja = pv_pool.tile([128, 384], F32, tag="pv")
            pjb = pv_pool.tile([128, 384], F32, tag="pv")
            pj = [pja, pjb]
            for h in range(H):
                for j in range(2):
                    nc.tensor.matmul(
                        pj[j], pooledT[:, h, qh * 128 : (qh + 1) * 128],
                        wout_sb[:, h, j * 384 : (j + 1) * 384],
                        start=(h == 0), stop=(h == H - 1),
                    )
            ot = work.tile([128, DM], F32, tag="ot")
            for j in range(2):
                sl = slice(j * 384, (j + 1) * 384)
                if ob_bc is not None:
                    nc.vector.tensor_add(out=ot[:, sl], in0=pj[j], in1=ob_bc[:, sl])
                else:
                    nc.vector.tensor_copy(out=ot[:, sl], in_=pj[j])
            nc.sync.dma_start(out=out[qh * 128 : (qh + 1) * 128, :], in_=ot)
    nc.compile()
    return nc


_NC_CACHE = {}
_TRACE = False


def kernel(**inputs):
    x = np.asarray(inputs["x"], dtype=np.float32)
    query = np.asarray(inputs["query"], dtype=np.float32)
    ln_k_g = np.asarray(inputs["ln_k_g"], dtype=np.float32)
    ln_k_b = np.asarray(inputs["ln_k_b"], dtype=np.float32)
    ln_q_g = np.asarray(inputs["ln_q_g"], dtype=np.float32)
    ln_q_b = np.asarray(inputs["ln_q_b"], dtype=np.float32)
    W_q = np.asarray(inputs["W_q"], dtype=np.float32)
    W_kv = np.asarray(inputs["W_kv"], dtype=np.float32)
    W_out = np.asarray(inputs["W_out"], dtype=np.float32)

    scale = DH ** -0.5
    wkvg = (ln_k_g[:, None] * W_kv).astype(np.float16)
    wqg = (ln_q_g[:, None] * W_q * scale).astype(np.float16)
    qbias = ((ln_q_b @ W_q) * scale).astype(np.float32)
    kv_bias = ln_k_b @ W_kv
    vb = kv_bias[DM:]
    has_out_bias = bool(np.any(vb != 0.0))
    x16 = x.astype(np.float16)

    key = has_out_bias
    if key not in _NC_CACHE:
        _NC_CACHE[key] = build_nc(has_out_bias)
    nc = _NC_CACHE[key]

    shared = dict(
        query=query.astype(np.float16), wkvg=wkvg, wqg=wqg,
        wout=W_out.astype(np.float16),
        qbias=qbias,
        identh=np.eye(128, dtype=np.float16),
        identf=np.eye(128, dtype=np.float32),
    )
    if has_out_bias:
        shared["out_bias"] = (vb @ W_out).astype(np.float32).reshape(1, DM)
    in_maps = [dict(x=x16[i], **shared) for i in range(N_CORES)]
    res = run_bass_kernel_spmd(
        nc, in_maps, core_ids=list(range(N_CORES)), trace=_TRACE
    )
    kernel.last_result = res
    out = np.stack([np.asarray(res.results[i]["out"]) for i in range(N_CORES)])
    return out.astype(np.float32)


if __name__ == "__main__":
    rng = np.random.default_rng(0)
    ins = {
        "x": rng.standard_normal((B, N, CTX), dtype=np.float32),
        "query": rng.standard_normal((NQ, DM), dtype=np.float32),
        "ln_k_g": np.ones(CTX, np.float32),
        "ln_k_b": np.zeros(CTX, np.float32),
        "ln_q_g": np.ones(DM, np.float32),
        "ln_q_b": np.zeros(DM, np.float32),
        "W_q": rng.standard_normal((DM, DM), dtype=np.float32) * DM ** -0.5,
        "W_kv": rng.standard_normal((CTX, 2 * DM), dtype=np.float32) * CTX ** -0.5,
        "W_out": rng.standard_normal((DM, DM), dtype=np.float32) * DM ** -0.5,
    }
    o = kernel(**ins)
    print("out", o.shape, o.dtype, float(np.abs(o).mean()))

